# revision 53
# baseline (speedup 1.0000x reference)
"""MLA prefill kernel for 8 trn2 NeuronCores.

Sharding: core c handles batch b = c//4, head group hg = c%4 (4 of 16 heads).
Each core computes its 4 heads' attention + its partial proj output
[T, C] (bf16); the host sums the 4 partials per batch (unshard of the
head-contracted proj output) and stacks batches.

Device dataflow per core (all matmul operands bf16, fp32 PSUM):
  A: q_nope^T [4x128, T] and rotated q_rope [t, 256] -> persistent SBUF;
     phase-B/C weights prefetch behind phase A's x/weight streams
  B: ckv [t, 576] -> rmsnorm(lat), rotate k_rope, PE-transpose to
     ckv'^T [4x128, T], kz zero-padded krope^T pair ([krope;0], [0;krope]
     so rope score matmuls contract a full 128 partitions), qropeT
  C: k_nope^T per head [128, T], v [128, 16, 512] -> persistent SBUF
  D: per (head, q-block 512): S^T tiles [128k, 512q] (nope+rope matmuls),
     exp via ACT from PSUM (masked subblocks via DVE STT), l via deferred
     ones-matmul burst (row-broadcast), att^T accum via V-matmul;
     normalize by exp(-ln l); proj accumulates 4 head-chunks; PSUM->SBUF
     drain split across DVE+ACT -> bf16 HBM store.
"""

import numpy as np

B, T, C, H = 2, 2048, 2048, 16
NOPE, ROPE, VD, LORA = 128, 64, 128, 512
QK = NOPE + ROPE
EPS = 1e-6
SCALE = 1.0 / float(np.sqrt(QK))
P = 128
KC = C // P    # 16 contraction chunks over C
TB = T // P    # 16 token sub-blocks
NQ = T // 512  # 4 q-blocks
HPC = 4        # heads per core
N_CORES = 8

_prog_cache = {}
LAST_RESULTS = None  # BassKernelResults of the most recent run (for test.py)
MM_DTYPE = "bfloat16"  # "bfloat16" or "float32r" for all matmul operands


# ---------------------------------------------------------------- host prep

def _mask_plan(mask):
    """Classify mask into per-(q-block, k-chunk) plans.

    plan[j] = list of (c, col0, subops); subops[qs] in
    {"skip", "zero", ("g", gidx)} for columns [128*qs, 128*qs+128) of the
    S^T tile. col0 = 128 * (# leading skip sub-blocks), forced to 0 for
    the first chunk of each j. Chunks with all sub-blocks skip are
    omitted (their softmax contribution is exactly 0 in fp32).
    """
    plan = []
    gblocks = []
    for j in range(NQ):
        chunks = []
        for c in range(TB):
            sub = []
            nskip_lead = 0
            leading = True
            any_alive = False
            for qs in range(4):
                blk = mask[512 * j + 128 * qs: 512 * j + 128 * qs + 128,
                           128 * c: 128 * c + 128]
                if np.all(blk <= -88.0):
                    sub.append("skip")
                    if leading:
                        nskip_lead += 1
                elif np.all(blk == 0.0):
                    sub.append("zero")
                    leading = False
                    any_alive = True
                else:
                    gidx = len(gblocks)
                    gblocks.append(np.ascontiguousarray(blk.T))
                    sub.append(("g", gidx))
                    leading = False
                    any_alive = True
            if not any_alive:
                continue
            col0 = 128 * nskip_lead
            if not chunks:
                col0 = 0  # first chunk must initialize full psum width
            chunks.append((c, col0, sub))
        assert chunks, f"q-block {j}: all keys masked (unsupported)"
        plan.append(chunks)
    if gblocks:
        garr = np.stack(gblocks).astype(np.float32)
    else:
        garr = np.zeros((1, 128, 128), np.float32)
    return plan, garr


def _plan_key(plan):
    return tuple(
        tuple((c, col0, tuple(s if isinstance(s, str) else ("g",) for s in sub))
              for (c, col0, sub) in chunks)
        for chunks in plan
    )


def _pack_wq(Wq, hg):
    """[C, 768]: 4 heads' nope cols, then 2 rope 'pair' chunks laid out
    [h_even(32) h_odd(32) h'_even(32) h'_odd(32)]."""
    heads = [4 * hg + i for i in range(HPC)]
    cols = [Wq[:, h * QK: h * QK + NOPE] for h in heads]
    for h in heads:
        rope = Wq[:, h * QK + NOPE: h * QK + QK]
        cols.append(rope[:, 0::2])
        cols.append(rope[:, 1::2])
    return np.ascontiguousarray(np.concatenate(cols, axis=1))


def _pack_wckv(Wckv):
    """[C, 576]: lat 512 | rope_even 32 | rope_odd 32."""
    lat = Wckv[:, :LORA]
    rope = Wckv[:, LORA:]
    return np.ascontiguousarray(
        np.concatenate([lat, rope[:, 0::2], rope[:, 1::2]], axis=1))


def _pack_wdkv(Wdkv, kv_norm_w, hg):
    """(wdn [LORA, 512], wdv [LORA, 512]) with kv_norm_w folded in."""
    Wd = Wdkv * kv_norm_w[:, None]
    heads = [4 * hg + i for i in range(HPC)]
    n_cols = [Wd[:, h * (NOPE + VD): h * (NOPE + VD) + NOPE] for h in heads]
    v_cols = [Wd[:, h * (NOPE + VD) + NOPE: (h + 1) * (NOPE + VD)]
              for h in heads]
    return (np.ascontiguousarray(np.concatenate(n_cols, axis=1)),
            np.ascontiguousarray(np.concatenate(v_cols, axis=1)))


# ---------------------------------------------------------------- program

def _act_tables_combined_only(arch):
    """Steer Bacc's ACT table chooser to the one set containing Exp+Ln+Copy
    so the kernel pays a single ~2.7us table load instead of thrashing
    between exp_and_others and natural_log on every softmax normalize."""
    from concourse.hw_specs import get_activation_tables
    mine = {"Exp", "Ln", "Copy", "Identity", "Square", "MemsetZero"}
    t = get_activation_tables(arch)
    out = {}
    for name, fns in t.items():
        if name == "natural_log_exp_and_others" or not any(
                f.name in mine for f in fns):
            out[name] = fns
        else:
            out[name] = set()
    return out


def _build(plan, n_generic):
    import concourse.mybir as mybir
    import concourse.tile as tile
    from concourse import bacc
    from concourse.masks import make_identity

    f32 = mybir.dt.float32
    f32r = getattr(mybir.dt, MM_DTYPE)
    AL = mybir.AluOpType
    AF = mybir.ActivationFunctionType

    def r(ap):
        return ap

    nc = bacc.Bacc(None, target_bir_lowering=False)

    xT_d = nc.dram_tensor("xT", [C, T], f32r, kind="ExternalInput")
    wq_d = nc.dram_tensor("wq", [C, 768], f32r, kind="ExternalInput")
    wckv_d = nc.dram_tensor("wckv", [C, 576], f32r, kind="ExternalInput")
    wdn_d = nc.dram_tensor("wdn", [LORA, 512], f32r, kind="ExternalInput")
    wdv_d = nc.dram_tensor("wdv", [LORA, 512], f32r, kind="ExternalInput")
    wproj_d = nc.dram_tensor("wproj", [512, C], f32r, kind="ExternalInput")
    cos_d = nc.dram_tensor("cosT", [T, 32], f32, kind="ExternalInput")
    sin_d = nc.dram_tensor("sinT", [T, 32], f32, kind="ExternalInput")
    maskg_d = nc.dram_tensor("maskg", [max(1, n_generic), 128, 128], f32,
                             kind="ExternalInput")
    out_d = nc.dram_tensor("out", [T, C], f32r, kind="ExternalOutput")

    xT_r = None  # set below (needs AP)

    with tile.TileContext(nc) as tc:
        with tc.tile_pool(name="const", bufs=1) as const, \
             tc.tile_pool(name="p1", bufs=1) as p1, \
             tc.tile_pool(name="pkn", bufs=1) as pkn:
            ident_f = const.tile([P, P], f32, tag="ident_f", name="ident_f")
            make_identity(nc, ident_f)
            ident = const.tile([P, P], f32r, tag="ident", name="ident")
            nc.scalar.copy(ident, ident_f)
            ones_f = const.tile([P, P], f32, tag="ones_f", name="ones_f")
            nc.any.memset(ones_f, 1.0)
            ones128 = const.tile([P, P], f32r, tag="ones", name="ones")
            nc.scalar.copy(ones128, ones_f)
            eps_t = const.tile([P, 1], f32, tag="eps", name="eps")
            nc.any.memset(eps_t, EPS)
            cos_all = const.tile([P, TB, 32], f32, tag="cos", name="cos")
            sin_all = const.tile([P, TB, 32], f32, tag="sin", name="sin")

            qropeT = [p1.tile([P, T], f32r, tag=f"qrT{i}", name=f"qrT{i}")
                      for i in range(2)]
            # zero-padded krope^T pair: kz[0] = [krope; 0], kz[1] =
            # [0; krope].  Rope score matmuls then contract a full 128
            # partitions (64-partition matmuls stream at only ~1.5
            # cycles/row); the zero half annihilates the other head's
            # q_rope rows in the shared moving operand.
            kz = [p1.tile([P, T], f32r, tag=f"kz{i}", name=f"kz{i}")
                  for i in range(2)]
            for i, half in ((0, slice(64, 128)), (1, slice(0, 64))):
                zv = kz[i][half, :]
                if MM_DTYPE == "float32r":
                    zv = zv.bitcast(f32)
                nc.any.memset(zv, 0.0)
            knopeT = [pkn.tile([P, T], f32r, tag=f"knT{h}", name=f"knT{h}")
                      for h in range(4)]
            # SBUF-resident intermediates (formerly DRAM scratch)
            qnopeT = [p1.tile([P, T], f32r, tag=f"qnT{h}", name=f"qnT{h}")
                      for h in range(4)]
            qrotS = p1.tile([P, TB, 256], f32r, tag="qrotS", name="qrotS")
            vS = p1.tile([P, TB, 512], f32r, tag="vS", name="vS")
            xT_r = xT_d[:].rearrange("(kc p) t -> p kc t", p=P)


            # shared PSUM pool for phases A..C: per-bank tags make bank
            # reuse a per-slot WAR dep instead of a pool-boundary barrier
            psu_cm = tc.tile_pool(name="psu", bufs=1, space="PSUM")
            psu = psu_cm.__enter__()

            # pools for phases A..C; pckvT/xb opened early so phase-B
            # weights and first x-slice can prefetch during phase A
            pckvT_cm = tc.tile_pool(name="pckvT", bufs=1)
            pckvT = pckvT_cm.__enter__()
            xb_cm = tc.tile_pool(name="xb", bufs=4)
            xb_pool = xb_cm.__enter__()
            wckv_sb = pckvT.tile([P, KC, 576], f32r, tag="wckv",
                                 name="wckv")
            wckv_r = wckv_d[:].rearrange("(kc p) m -> p kc m", p=P)
            ckvT = [pckvT.tile([P, T], f32r, tag=f"ckvT{d}",
                               name=f"ckvT{d}") for d in range(4)]
            wdn = pckvT.tile([P, 4, 512], f32r, tag="wdn", name="wdn")
            wdv = pckvT.tile([P, 4, 512], f32r, tag="wdv", name="wdv")

            def load_xb(n):
                xbh = []
                for hf in range(2):
                    xb = xb_pool.tile([P, 8, 256], f32r, tag="xb",
                                      name="xb")
                    nc.sync.dma_start(
                        xb, xT_r[:, 8 * hf:8 * hf + 8,
                                 n * 256:(n + 1) * 256])
                    xbh.append(xb)
                return xbh

            xb_pre = None

            # ---- phase A: q_nope^T (SBUF) and rotated q_rope (SBUF) -----
            with tc.tile_pool(name="phA", bufs=1) as pA, \
                 tc.tile_pool(name="xa", bufs=4) as xa_pool, \
                 tc.tile_pool(name="stA", bufs=3) as stA:
                wqn = pA.tile([P, KC, 512], f32r, tag="wqn", name="wqn")
                wqr = pA.tile([P, KC, 256], f32r, tag="wqr", name="wqr")
                wq_r = wq_d[:].rearrange("(kc p) m -> p kc m", p=P)
                # first chunks split finely so early matmuls start ASAP
                nc.sync.dma_start(wqn[:, 0:1], wq_r[:, 0:1, 0:512])
                nc.sync.dma_start(wqn[:, 1:2], wq_r[:, 1:2, 0:512])
                nc.sync.dma_start(wqn[:, 2:4], wq_r[:, 2:4, 0:512])

                for n in range(4):
                    pn = [psu.tile([P, 512], f32, tag=f"bk{m}",
                                   name=f"qn{m}") for m in range(4)]
                    pr = [psu.tile([P, 256], f32, tag=f"bk{4 + s}",
                                   name=f"qr{s}") for s in range(4)]
                    # bulk DMAs fetch all 16 xT chunks for this t-slice;
                    # each psum group then runs 16 back-to-back matmuls
                    xah = []
                    for hf in range(2):
                        xa = xa_pool.tile([P, 8, 512], f32r, tag="xa",
                                          name="xa")
                        if n == 0 and hf == 0:
                            # split: chunk 0 lands first for matmul 0
                            nc.sync.dma_start(
                                xa[:, 0:1], xT_r[:, 0:1, 0:512])
                            nc.sync.dma_start(
                                xa[:, 1:2], xT_r[:, 1:2, 0:512])
                            nc.sync.dma_start(
                                xa[:, 2:8], xT_r[:, 2:8, 0:512])
                        else:
                            nc.sync.dma_start(
                                xa, xT_r[:, 8 * hf:8 * hf + 8,
                                         n * 512:(n + 1) * 512])
                        xah.append(xa)

                    if n == 0:
                        # remaining weights queue behind the first-matmul
                        # critical loads, ordered by first compute use
                        for qk in range(1, 4):
                            nc.sync.dma_start(
                                wqn[:, 4 * qk:4 * qk + 4],
                                wq_r[:, 4 * qk:4 * qk + 4, 0:512])
                        nc.sync.dma_start(wqr, wq_r[:, :, 512:768])
                        nc.sync.dma_start(
                            cos_all,
                            cos_d[:].rearrange("(tb p) i -> p tb i", p=P))
                        nc.sync.dma_start(
                            sin_all,
                            sin_d[:].rearrange("(tb p) i -> p tb i", p=P))
                    if n == 1:
                        # phase-B weights prefetch behind phase-A's
                        for wk in range(4):
                            nc.sync.dma_start(
                                wckv_sb[:, 4 * wk:4 * wk + 4],
                                wckv_r[:, 4 * wk:4 * wk + 4])
                    if n == 2:
                        # phase-C weights, needed later still
                        nc.sync.dma_start(
                            wdn, wdn_d[:].rearrange("(kc p) m -> p kc m",
                                                    p=P))
                        nc.sync.dma_start(
                            wdv, wdv_d[:].rearrange("(kc p) m -> p kc m",
                                                    p=P))
                    if n == 3:
                        # phase B's first x-slice overlaps phase-A tail
                        xb_pre = load_xb(0)

                    def xat(k):
                        return xah[k // 8][:, k % 8]

                    for m in range(4):
                        for k in range(KC):
                            nc.tensor.matmul(
                                pn[m], r(wqn[:, k, m * 128:(m + 1) * 128]),
                                r(xat(k)), start=(k == 0), stop=(k == KC - 1))
                    for s in range(4):
                        for k in range(KC):
                            nc.tensor.matmul(
                                pr[s], r(xat(k)[:, s * 128:(s + 1) * 128]),
                                r(wqr[:, k, :]),
                                start=(k == 0), stop=(k == KC - 1))
                    for m in range(4):
                        nc.scalar.copy(
                            qnopeT[m][:, n * 512:(n + 1) * 512], pn[m])
                    for s in range(4):
                        tb = n * 4 + s
                        cosv = cos_all[:, tb][:, None, :].to_broadcast(
                            (P, 4, 32))
                        sinv = sin_all[:, tb][:, None, :].to_broadcast(
                            (P, 4, 32))
                        prv = pr[s].rearrange("p (g i) -> p g i", i=64)
                        qe, qo = prv[:, :, 0:32], prv[:, :, 32:64]
                        ta = stA.tile([P, 128], f32, tag="ta",
                                      name="ta").rearrange(
                            "p (g i) -> p g i", i=32)
                        tb_ = stA.tile([P, 128], f32, tag="tb",
                                       name="tb").rearrange(
                            "p (g i) -> p g i", i=32)
                        tc2 = stA.tile([P, 128], f32, tag="tc",
                                       name="tc").rearrange(
                            "p (g i) -> p g i", i=32)
                        td = stA.tile([P, 128], f32, tag="td",
                                      name="td").rearrange(
                            "p (g i) -> p g i", i=32)
                        qvv = qrotS[:, tb].rearrange("p (g i) -> p g i",
                                                     i=64)
                        nc.vector.tensor_tensor(ta, qe, cosv, AL.mult)
                        nc.vector.tensor_tensor(tb_, qo, sinv, AL.mult)
                        nc.vector.tensor_tensor(qvv[:, :, 0:32], ta, tb_,
                                                AL.subtract)
                        nc.vector.tensor_tensor(tc2, qo, cosv, AL.mult)
                        nc.vector.tensor_tensor(td, qe, sinv, AL.mult)
                        nc.vector.tensor_tensor(qvv[:, :, 32:64], tc2, td,
                                                AL.add)

            # ---- phase B: ckv -> rms/rope -> transposed tensors ----------
            if True:
                with tc.tile_pool(name="stB", bufs=3) as stB, \
                     tc.tile_pool(name="smB", bufs=4) as smB:
                    tr_idx = [0]
                    for n in range(8):  # 256-token slices
                        pcs = [[psu.tile([P, 288], f32,
                                         tag=f"bk{2 * s_ + u}",
                                         name=f"ckv{u}")
                                for u in range(2)] for s_ in range(2)]
                        xbh = xb_pre if n == 0 else load_xb(n)

                        def xbt(k):
                            return xbh[k // 8][:, k % 8]

                        for s in range(2):
                            for u in range(2):
                                wsl = (slice(0, 288), slice(288, 576))[u]
                                for k in range(KC):
                                    nc.tensor.matmul(
                                        pcs[s][u],
                                        r(xbt(k)[:, s * 128:(s + 1) * 128]),
                                        r(wckv_sb[:, k, wsl]),
                                        start=(k == 0), stop=(k == KC - 1))
                        for s in range(2):
                            tb = n * 2 + s
                            p0, p1_ = pcs[s]
                            sq = stB.tile([P, 288], f32, tag="sq", name="sq")
                            sq2 = stB.tile([P, 224], f32, tag="sq2",
                                           name="sq2")
                            ss0 = smB.tile([P, 1], f32, tag="ss0", name="ss0")
                            ss1 = smB.tile([P, 1], f32, tag="ss1", name="ss1")
                            nc.scalar.activation(sq, p0, AF.Square,
                                                 accum_out=ss0)
                            nc.scalar.activation(sq2, p1_[:, 0:224],
                                                 AF.Square, accum_out=ss1)
                            ssum = smB.tile([P, 1], f32, tag="ss", name="ss")
                            nc.vector.tensor_add(ssum, ss0, ss1)
                            lnv = smB.tile([P, 1], f32, tag="lnv", name="lnv")
                            nc.scalar.activation(lnv, ssum, AF.Ln,
                                                 bias=eps_t,
                                                 scale=1.0 / LORA)
                            rfac = smB.tile([P, 1], f32, tag="rfac",
                                            name="rfac")
                            nc.scalar.activation(rfac, lnv, AF.Exp,
                                                 scale=-0.5)
                            ckvn = stB.tile([P, 512], f32r, tag="ckvn",
                                            name="ckvn")
                            nc.scalar.mul(ckvn[:, 0:288], p0, rfac)
                            nc.scalar.mul(ckvn[:, 288:512], p1_[:, 0:224],
                                          rfac)
                            # k_rope rotation (raw latent, un-normalized)
                            ke, ko = p1_[:, 224:256], p1_[:, 256:288]
                            cosv, sinv = cos_all[:, tb], sin_all[:, tb]
                            ra = stB.tile([P, 32], f32, tag="ra", name="ra")
                            rb = stB.tile([P, 32], f32, tag="rb", name="rb")
                            rc = stB.tile([P, 32], f32, tag="rc", name="rc")
                            rd = stB.tile([P, 32], f32, tag="rd", name="rd")
                            krt = stB.tile([P, 64], f32r, tag="krt",
                                           name="krt")
                            nc.vector.tensor_tensor(ra, ke, cosv, AL.mult)
                            nc.vector.tensor_tensor(rb, ko, sinv, AL.mult)
                            nc.vector.tensor_tensor(krt[:, 0:32], ra, rb,
                                                    AL.subtract)
                            nc.vector.tensor_tensor(rc, ko, cosv, AL.mult)
                            nc.vector.tensor_tensor(rd, ke, sinv, AL.mult)
                            nc.vector.tensor_tensor(krt[:, 32:64], rc, rd,
                                                    AL.add)
                            # transposes -> persistent ^T tensors
                            tcol = slice(tb * 128, (tb + 1) * 128)
                            for dc in range(4):
                                pt = psu.tile([P, P], f32r,
                                              tag=f"bk{4 + tr_idx[0] % 4}",
                                              name="tr")
                                tr_idx[0] += 1
                                nc.tensor.transpose(
                                    pt, ckvn[:, dc * 128:(dc + 1) * 128],
                                    ident)
                                nc.vector.tensor_copy(ckvT[dc][:, tcol], pt)
                            pt = psu.tile([P, P], f32r,
                                          tag=f"bk{4 + tr_idx[0] % 4}",
                                          name="tr")
                            tr_idx[0] += 1
                            nc.tensor.transpose(pt[0:64, :], krt, ident)
                            nc.vector.tensor_copy(kz[0][0:64, tcol],
                                                  pt[0:64, :])
                            qr_in = qrotS[:, tb]
                            for pc in range(2):
                                pt = psu.tile([P, P], f32r,
                                              tag=f"bk{4 + tr_idx[0] % 4}",
                                              name="tr")
                                tr_idx[0] += 1
                                nc.tensor.transpose(
                                    pt, qr_in[:, pc * 128:(pc + 1) * 128],
                                    ident)
                                nc.vector.tensor_copy(qropeT[pc][:, tcol], pt)

                # odd heads' row-group: shifted copy into the upper half
                nc.sync.dma_start(kz[1][64:128, :], kz[0][0:64, :])

                # phase-D mask tiles prefetch while phase C computes
                mg_all = {}
                for g in range(n_generic):
                    mt = const.tile([P, P], f32, tag=f"mg{g}", name=f"mg{g}")
                    nc.sync.dma_start(mt, maskg_d[g])
                    mg_all[g] = mt

                # ---- phase C: k_nope^T per head, v (SBUF) ---------------
                for h in range(4):
                    for n4 in range(4):
                        pk = psu.tile([P, 512], f32,
                                      tag=f"bk{(h * 4 + n4) % 4}",
                                      name="kn")
                        for kc in range(4):
                            nc.tensor.matmul(
                                pk, r(wdn[:, kc, h * 128:(h + 1) * 128]),
                                r(ckvT[kc][:, n4 * 512:(n4 + 1) * 512]),
                                start=(kc == 0), stop=(kc == 3))
                        nc.vector.tensor_copy(
                            knopeT[h][:, n4 * 512:(n4 + 1) * 512], pk)
                for tb in range(TB):
                    pv = psu.tile([P, 512], f32,
                                  tag=f"bk{4 + tb % 4}", name="v")
                    for kc in range(4):
                        nc.tensor.matmul(
                            pv, r(ckvT[kc][:, tb * P:(tb + 1) * P]),
                            r(wdv[:, kc, :]),
                            start=(kc == 0), stop=(kc == 3))
                    nc.vector.tensor_copy(vS[:, tb], pv)

            xb_cm.__exit__(None, None, None)
            pckvT_cm.__exit__(None, None, None)
            psu_cm.__exit__(None, None, None)

            # ---- phase D: attention + proj -------------------------------
            with tc.tile_pool(name="phD", bufs=1) as pD, \
                 tc.tile_pool(name="sp", bufs=18) as sp, \
                 tc.tile_pool(name="stD", bufs=3) as stD, \
                 tc.tile_pool(name="attp", bufs=2) as attp, \
                 tc.tile_pool(name="psS", bufs=3, space="PSUM") as psS, \
                 tc.tile_pool(name="psAtt", bufs=2, space="PSUM") as psAtt, \
                 tc.tile_pool(name="psL", bufs=1, space="PSUM") as psL, \
                 tc.tile_pool(name="psO", bufs=2, space="PSUM") as psO:
                wproj_sb = [pD.tile([P, C], f32r, tag=f"wp{h}",
                                    name=f"wp{h}") for h in range(4)]
                for h in range(4):
                    nc.sync.dma_start(wproj_sb[h],
                                      wproj_d[h * P:(h + 1) * P, :])
                def emit_proj(jp, att_p):
                    for qs in range(4):
                        for ct in range(4):
                            pso = psO.tile([P, 512], f32, tag="o",
                                            name="o")
                            for h in range(4):
                                nc.tensor.matmul(
                                    pso,
                                    r(att_p[h][:, qs * 128:(qs + 1) * 128]),
                                    r(wproj_sb[h][:,
                                                  ct * 512:(ct + 1) * 512]),
                                    start=(h == 0), stop=(h == 3))
                            ost = sp.tile([P, 512], f32r, tag="ost",
                                          name="ost", bufs=3)
                            # split PSUM read across both engines so the
                            # bank frees sooner for the next proj group
                            nc.vector.tensor_copy(ost[:, 0:256],
                                                  pso[:, 0:256])
                            nc.scalar.copy(ost[:, 256:512],
                                           pso[:, 256:512])
                            nc.sync.dma_start(
                                out_d[512 * jp + 128 * qs:
                                      512 * jp + 128 * (qs + 1),
                                      ct * 512:(ct + 1) * 512], ost)

                prev_proj = None
                for j in range(NQ):
                    chunks = plan[j]
                    nchunks = len(chunks)
                    attT = {}
                    for g in range(4):  # heads, pipelined sequentially
                        hs = (g,)
                        st = {}
                        for h in hs:
                            ps_att = psAtt.tile([P, 512], f32, tag="att",
                                                name="att")
                            ps_l = psL.tile([P, 512], f32, tag="l",
                                             name="l")
                            st[h] = dict(att=ps_att, l=ps_l)

                        def scores_mm(h, ci):
                            c, col0, sub = chunks[ci]
                            hp, pr_ = h % 2, h // 2
                            qsl = slice(512 * j + col0, 512 * (j + 1))
                            kcl = slice(128 * c, 128 * (c + 1))
                            ps_s = psS.tile([P, 512], f32, tag="s", name="s")
                            nc.tensor.matmul(
                                ps_s[:, col0:], r(knopeT[h][:, kcl]),
                                r(qnopeT[h][:, 512 * j + col0:
                                            512 * (j + 1)]),
                                start=True, stop=False)
                            nc.tensor.matmul(
                                ps_s[:, col0:],
                                r(kz[hp][:, kcl]),
                                r(qropeT[pr_][:, qsl]),
                                start=False, stop=True)
                            return ps_s

                        def exp_mask(h, ci, ps_s):
                            c, col0, sub = chunks[ci]
                            sprime = sp.tile([P, 512], f32r, tag="sp",
                                             name="sp")
                            nc.scalar.activation(
                                sprime[:, col0:], ps_s[:, col0:],
                                AF.Exp, scale=SCALE)
                            for qs, s in enumerate(sub):
                                colA, colB = 128 * qs, 128 * (qs + 1)
                                if colA < col0 or s == "zero":
                                    continue
                                if s == "skip":
                                    zv = sprime[:, colA:colB]
                                    if MM_DTYPE == "float32r":
                                        zv = zv.bitcast(f32)
                                    nc.any.memset(zv, 0.0)
                                else:
                                    mt = mg_all[s[1]]
                                    stt = stD.tile([P, P], f32, tag="stt",
                                                   name="stt")
                                    nc.vector.scalar_tensor_tensor(
                                        stt, ps_s[:, colA:colB], SCALE, mt,
                                        AL.mult, AL.add)
                                    nc.scalar.activation(
                                        sprime[:, colA:colB], stt, AF.Exp,
                                        scale=1.0)
                            return sprime

                        def att_mm(h, ci, sprime):
                            c, col0, sub = chunks[ci]
                            nc.tensor.matmul(
                                st[h]["att"][:, col0:],
                                r(vS[:, c, h * 128:(h + 1) * 128]),
                                r(sprime[:, col0:]),
                                start=(ci == 0), stop=(ci == nchunks - 1))

                        h = hs[0]
                        pend = [scores_mm(h, 0)]
                        if nchunks > 1:
                            pend.append(scores_mm(h, 1))
                        sprimes = []
                        for ci in range(nchunks):
                            sprime = exp_mask(h, ci, pend[ci])
                            sprimes.append(sprime)
                            if ci + 2 < nchunks:
                                pend.append(scores_mm(h, ci + 2))
                            att_mm(h, ci, sprime)
                        # deferred row-sum burst: dense back-to-back matmuls,
                        # no per-chunk ACT waits on the PE stream
                        for ci, sprime in enumerate(sprimes):
                            col0 = chunks[ci][1]
                            nc.tensor.matmul(
                                st[h]["l"][:, col0:], r(ones128),
                                r(sprime[:, col0:]),
                                start=(ci == 0), stop=(ci == nchunks - 1))
                        for h in hs:
                            lnl = stD.tile([P, 512], f32, tag="lr",
                                           name="lr")
                            nc.scalar.activation(lnl, st[h]["l"], AF.Ln)
                            rec = stD.tile([P, 512], f32, tag="lr",
                                           name="lr")
                            nc.scalar.activation(rec, lnl, AF.Exp,
                                                 scale=-1.0)
                            at = attp.tile([P, 512], f32r, tag=f"at{h}",
                                           name=f"at{h}")
                            nc.vector.tensor_tensor(at, st[h]["att"], rec,
                                                    AL.mult)
                            attT[h] = at
                    emit_proj(j, attT)

    orig_tables = bacc.get_activation_tables
    bacc.get_activation_tables = _act_tables_combined_only
    try:
        nc.compile()
    finally:
        bacc.get_activation_tables = orig_tables
    return nc


# ---------------------------------------------------------------- entry

def _ensure_axon_hook_shim():
    # bass_utils imports antenv.axon_hooks when tracing is requested via
    # env; provide a null hook module if the image lacks it so kernel()
    # never crashes on that path.
    try:
        import antenv.axon_hooks  # noqa: F401
    except Exception:
        import sys
        import types
        m = types.ModuleType("antenv.axon_hooks")
        _h = [None]
        m.set_axon_ntff_profile_hook = lambda h: _h.__setitem__(0, h)
        m.get_axon_ntff_profile_hook = lambda: _h[0]
        sys.modules["antenv.axon_hooks"] = m
        try:
            import antenv
            antenv.axon_hooks = m
        except Exception:
            pass


def kernel(x, freq_cis, mask, window, Wq, Wckv, kv_norm_w, Wdkv, Wproj,
           start_pos):
    global LAST_RESULTS
    _ensure_axon_hook_shim()
    from concourse.bass_utils import run_bass_kernel_spmd

    x = np.asarray(x, np.float32)
    freq_cis = np.asarray(freq_cis, np.float32)
    mask = np.asarray(mask, np.float32)
    Wq = np.asarray(Wq, np.float32)
    Wckv = np.asarray(Wckv, np.float32)
    kv_norm_w = np.asarray(kv_norm_w, np.float32)
    Wdkv = np.asarray(Wdkv, np.float32)
    Wproj = np.asarray(Wproj, np.float32)

    plan, maskg = _mask_plan(mask)
    key = (MM_DTYPE, _plan_key(plan))
    if key not in _prog_cache:
        _prog_cache[key] = _build(plan, maskg.shape[0])
    nc = _prog_cache[key]

    cosT = np.ascontiguousarray(freq_cis[:, :, 0])
    sinT = np.ascontiguousarray(freq_cis[:, :, 1])
    wckv_p = _pack_wckv(Wckv)

    in_maps = []
    for core in range(N_CORES):
        b, hg = core // 4, core % 4
        wdn, wdv = _pack_wdkv(Wdkv, kv_norm_w, hg)
        in_maps.append({
            "xT": np.ascontiguousarray(x[b].T),
            "wq": _pack_wq(Wq, hg),
            "wckv": wckv_p,
            "wdn": wdn,
            "wdv": wdv,
            "wproj": np.ascontiguousarray(Wproj[hg * 512:(hg + 1) * 512, :]),
            "cosT": cosT,
            "sinT": sinT,
            "maskg": maskg,
        })

    if MM_DTYPE == "bfloat16":
        import ml_dtypes
        mmdt = ml_dtypes.bfloat16
        for m in in_maps:
            for k in ("xT", "wq", "wckv", "wdn", "wdv", "wproj"):
                m[k] = m[k].astype(mmdt)

    res = run_bass_kernel_spmd(nc, in_maps, list(range(N_CORES)))
    LAST_RESULTS = res
    outs = [np.asarray(res.results[c]["out"], np.float32)
            for c in range(N_CORES)]
    full = np.empty((B, T, C), np.float32)
    for b in range(B):
        full[b] = outs[4 * b] + outs[4 * b + 1] + outs[4 * b + 2] \
            + outs[4 * b + 3]
    return full



# revision 55
# speedup vs baseline: 1.1857x; 1.1857x over previous
"""MLA prefill kernel for 8 trn2 NeuronCores.

Sharding: core c handles batch b = c//4, head group hg = c%4 (4 of 16 heads).
Each core computes its 4 heads' attention + its partial proj output
[T, C] (bf16); the host sums the 4 partials per batch (unshard of the
head-contracted proj output) and stacks batches.

Device dataflow per core (all matmul operands bf16, fp32 PSUM):
  A: q_nope^T [4x128, T] and rotated q_rope [t, 256] -> persistent SBUF;
     phase-B/C weights prefetch behind phase A's x/weight streams
  B: ckv [t, 576] -> rmsnorm(lat), rotate k_rope, PE-transpose to
     ckv'^T [4x128, T], kz zero-padded krope^T pair ([krope;0], [0;krope]
     so rope score matmuls contract a full 128 partitions), qropeT
  C: k_nope^T per head [128, T], v [128, 16, 512] -> persistent SBUF
  D: per (head, q-block 512): S^T tiles [128k, 512q] (nope+rope matmuls),
     exp via ACT from PSUM (masked subblocks via DVE STT), l via deferred
     ones-matmul burst (row-broadcast), att^T accum via V-matmul;
     normalize by exp(-ln l); proj accumulates 4 head-chunks; PSUM->SBUF
     drain split across DVE+ACT -> bf16 HBM store.
"""

import numpy as np

B, T, C, H = 2, 2048, 2048, 16
NOPE, ROPE, VD, LORA = 128, 64, 128, 512
QK = NOPE + ROPE
EPS = 1e-6
SCALE = 1.0 / float(np.sqrt(QK))
P = 128
KC = C // P    # 16 contraction chunks over C
TB = T // P    # 16 token sub-blocks
NQ = T // 512  # 4 q-blocks
HPC = 4        # heads per core
N_CORES = 8

_prog_cache = {}
LAST_RESULTS = None  # BassKernelResults of the most recent run (for test.py)
MM_DTYPE = "bfloat16"  # "bfloat16" or "float32r" for all matmul operands


# ---------------------------------------------------------------- host prep

def _mask_plan(mask):
    """Classify mask into per-(q-block, k-chunk) plans.

    plan[j] = list of (c, col0, subops); subops[qs] in
    {"skip", "zero", ("g", gidx)} for columns [128*qs, 128*qs+128) of the
    S^T tile. col0 = 128 * (# leading skip sub-blocks), forced to 0 for
    the first chunk of each j. Chunks with all sub-blocks skip are
    omitted (their softmax contribution is exactly 0 in fp32).
    """
    plan = []
    gblocks = []
    for j in range(NQ):
        chunks = []
        for c in range(TB):
            sub = []
            nskip_lead = 0
            leading = True
            any_alive = False
            for qs in range(4):
                blk = mask[512 * j + 128 * qs: 512 * j + 128 * qs + 128,
                           128 * c: 128 * c + 128]
                if np.all(blk <= -88.0):
                    sub.append("skip")
                    if leading:
                        nskip_lead += 1
                elif np.all(blk == 0.0):
                    sub.append("zero")
                    leading = False
                    any_alive = True
                else:
                    gidx = len(gblocks)
                    gblocks.append(np.ascontiguousarray(blk.T))
                    sub.append(("g", gidx))
                    leading = False
                    any_alive = True
            if not any_alive:
                continue
            col0 = 128 * nskip_lead
            if not chunks:
                col0 = 0  # first chunk must initialize full psum width
            chunks.append((c, col0, sub))
        assert chunks, f"q-block {j}: all keys masked (unsupported)"
        plan.append(chunks)
    if gblocks:
        garr = np.stack(gblocks).astype(np.float32)
    else:
        garr = np.zeros((1, 128, 128), np.float32)
    return plan, garr


def _plan_key(plan):
    return tuple(
        tuple((c, col0, tuple(s if isinstance(s, str) else ("g",) for s in sub))
              for (c, col0, sub) in chunks)
        for chunks in plan
    )


def _pack_wq(Wq, hg):
    """[C, 768]: 4 heads' nope cols, then 2 rope 'pair' chunks laid out
    [h_even(32) h_odd(32) h'_even(32) h'_odd(32)]."""
    heads = [4 * hg + i for i in range(HPC)]
    cols = [Wq[:, h * QK: h * QK + NOPE] for h in heads]
    for h in heads:
        rope = Wq[:, h * QK + NOPE: h * QK + QK]
        cols.append(rope[:, 0::2])
        cols.append(rope[:, 1::2])
    return np.ascontiguousarray(np.concatenate(cols, axis=1))


def _pack_wckv(Wckv):
    """[C, 576]: lat 512 | rope_even 32 | rope_odd 32."""
    lat = Wckv[:, :LORA]
    rope = Wckv[:, LORA:]
    return np.ascontiguousarray(
        np.concatenate([lat, rope[:, 0::2], rope[:, 1::2]], axis=1))


def _pack_wdkv(Wdkv, kv_norm_w, hg):
    """(wdn [LORA, 512], wdv [LORA, 512]) with kv_norm_w folded in."""
    Wd = Wdkv * kv_norm_w[:, None]
    heads = [4 * hg + i for i in range(HPC)]
    n_cols = [Wd[:, h * (NOPE + VD): h * (NOPE + VD) + NOPE] for h in heads]
    v_cols = [Wd[:, h * (NOPE + VD) + NOPE: (h + 1) * (NOPE + VD)]
              for h in heads]
    return (np.ascontiguousarray(np.concatenate(n_cols, axis=1)),
            np.ascontiguousarray(np.concatenate(v_cols, axis=1)))


# ---------------------------------------------------------------- program

def _act_tables_combined_only(arch):
    """Steer Bacc's ACT table chooser to the one set containing Exp+Ln+Copy
    so the kernel pays a single ~2.7us table load instead of thrashing
    between exp_and_others and natural_log on every softmax normalize."""
    from concourse.hw_specs import get_activation_tables
    mine = {"Exp", "Ln", "Copy", "Identity", "Square", "MemsetZero"}
    t = get_activation_tables(arch)
    out = {}
    for name, fns in t.items():
        if name == "natural_log_exp_and_others" or not any(
                f.name in mine for f in fns):
            out[name] = fns
        else:
            out[name] = set()
    return out


def _build(plan, n_generic):
    import concourse.mybir as mybir
    import concourse.tile as tile
    from concourse import bacc
    from concourse.masks import make_identity

    f32 = mybir.dt.float32
    f32r = getattr(mybir.dt, MM_DTYPE)
    AL = mybir.AluOpType
    AF = mybir.ActivationFunctionType

    def r(ap):
        return ap

    nc = bacc.Bacc(None, target_bir_lowering=False)

    xT_d = nc.dram_tensor("xT", [C, T], f32r, kind="ExternalInput")
    wq_d = nc.dram_tensor("wq", [C, 768], f32r, kind="ExternalInput")
    wckv_d = nc.dram_tensor("wckv", [C, 576], f32r, kind="ExternalInput")
    wdn_d = nc.dram_tensor("wdn", [LORA, 512], f32r, kind="ExternalInput")
    wdv_d = nc.dram_tensor("wdv", [LORA, 512], f32r, kind="ExternalInput")
    wproj_d = nc.dram_tensor("wproj", [512, C], f32r, kind="ExternalInput")
    cos_d = nc.dram_tensor("cosT", [T, 32], f32, kind="ExternalInput")
    sin_d = nc.dram_tensor("sinT", [T, 32], f32, kind="ExternalInput")
    maskg_d = nc.dram_tensor("maskg", [max(1, n_generic), 128, 128], f32,
                             kind="ExternalInput")
    out_d = nc.dram_tensor("out", [T, C], f32r, kind="ExternalOutput")

    xT_r = None  # set below (needs AP)

    with tile.TileContext(nc) as tc:
        with tc.tile_pool(name="const", bufs=1) as const, \
             tc.tile_pool(name="p1", bufs=1) as p1, \
             tc.tile_pool(name="pkn", bufs=1) as pkn:
            ident_f = const.tile([P, P], f32, tag="ident_f", name="ident_f")
            make_identity(nc, ident_f)
            ident = const.tile([P, P], f32r, tag="ident", name="ident")
            nc.scalar.copy(ident, ident_f)
            ones_f = const.tile([P, P], f32, tag="ones_f", name="ones_f")
            nc.any.memset(ones_f, 1.0)
            ones128 = const.tile([P, P], f32r, tag="ones", name="ones")
            nc.scalar.copy(ones128, ones_f)
            eps_t = const.tile([P, 1], f32, tag="eps", name="eps")
            nc.any.memset(eps_t, EPS)
            cos_all = const.tile([P, TB, 32], f32, tag="cos", name="cos")
            sin_all = const.tile([P, TB, 32], f32, tag="sin", name="sin")

            qropeT = [p1.tile([P, T], f32r, tag=f"qrT{i}", name=f"qrT{i}")
                      for i in range(2)]
            # zero-padded krope^T pair: kz[0] = [krope; 0], kz[1] =
            # [0; krope].  Rope score matmuls then contract a full 128
            # partitions (64-partition matmuls stream at only ~1.5
            # cycles/row); the zero half annihilates the other head's
            # q_rope rows in the shared moving operand.
            kz = [p1.tile([P, T], f32r, tag=f"kz{i}", name=f"kz{i}")
                  for i in range(2)]
            for i, half in ((0, slice(64, 128)), (1, slice(0, 64))):
                zv = kz[i][half, :]
                if MM_DTYPE == "float32r":
                    zv = zv.bitcast(f32)
                nc.any.memset(zv, 0.0)
            knopeT = [pkn.tile([P, T], f32r, tag=f"knT{h}", name=f"knT{h}")
                      for h in range(4)]
            # SBUF-resident intermediates (formerly DRAM scratch)
            qnopeT = [p1.tile([P, T], f32r, tag=f"qnT{h}", name=f"qnT{h}")
                      for h in range(4)]
            qrotS = p1.tile([P, TB, 256], f32r, tag="qrotS", name="qrotS")
            vS = p1.tile([P, TB, 512], f32r, tag="vS", name="vS")
            xT_r = xT_d[:].rearrange("(kc p) t -> p kc t", p=P)


            # shared PSUM pool for phases A..C: per-bank tags make bank
            # reuse a per-slot WAR dep instead of a pool-boundary barrier
            psu_cm = tc.tile_pool(name="psu", bufs=1, space="PSUM")
            psu = psu_cm.__enter__()

            # pools for phases A..C; pckvT/xb opened early so phase-B
            # weights and first x-slice can prefetch during phase A
            pckvT_cm = tc.tile_pool(name="pckvT", bufs=1)
            pckvT = pckvT_cm.__enter__()
            xb_cm = tc.tile_pool(name="xb", bufs=4)
            xb_pool = xb_cm.__enter__()
            wckv_sb = pckvT.tile([P, KC, 576], f32r, tag="wckv",
                                 name="wckv")
            wckv_r = wckv_d[:].rearrange("(kc p) m -> p kc m", p=P)
            ckvT = [pckvT.tile([P, T], f32r, tag=f"ckvT{d}",
                               name=f"ckvT{d}") for d in range(4)]
            wdn = pckvT.tile([P, 4, 512], f32r, tag="wdn", name="wdn")
            wdv = pckvT.tile([P, 4, 512], f32r, tag="wdv", name="wdv")

            def load_xb(n):
                xbh = []
                for hf in range(2):
                    xb = xb_pool.tile([P, 8, 256], f32r, tag="xb",
                                      name="xb")
                    nc.sync.dma_start(
                        xb, xT_r[:, 8 * hf:8 * hf + 8,
                                 n * 256:(n + 1) * 256])
                    xbh.append(xb)
                return xbh

            xb_pre = None

            # ---- phase A: q_nope^T (SBUF) and rotated q_rope (SBUF) -----
            with tc.tile_pool(name="phA", bufs=1) as pA, \
                 tc.tile_pool(name="xa", bufs=4) as xa_pool, \
                 tc.tile_pool(name="stA", bufs=3) as stA:
                wqn = pA.tile([P, KC, 512], f32r, tag="wqn", name="wqn")
                wqr = pA.tile([P, KC, 256], f32r, tag="wqr", name="wqr")
                wq_r = wq_d[:].rearrange("(kc p) m -> p kc m", p=P)
                # first chunk alone so matmul 0 can start ASAP
                nc.sync.dma_start(wqn[:, 0:1], wq_r[:, 0:1, 0:512])
                nc.sync.dma_start(wqn[:, 1:4], wq_r[:, 1:4, 0:512])

                for n in range(4):
                    pn = [psu.tile([P, 512], f32, tag=f"bk{m}",
                                   name=f"qn{m}") for m in range(4)]
                    pr = [psu.tile([P, 256], f32, tag=f"bk{4 + s}",
                                   name=f"qr{s}") for s in range(4)]
                    # bulk DMAs fetch all 16 xT chunks for this t-slice;
                    # each psum group then runs 16 back-to-back matmuls
                    xah = []
                    for hf in range(2):
                        xa = xa_pool.tile([P, 8, 512], f32r, tag="xa",
                                          name="xa")
                        if n == 0 and hf == 0:
                            # split: chunk 0 lands first for matmul 0
                            nc.sync.dma_start(
                                xa[:, 0:1], xT_r[:, 0:1, 0:512])
                            nc.sync.dma_start(
                                xa[:, 1:8], xT_r[:, 1:8, 0:512])
                        else:
                            nc.sync.dma_start(
                                xa, xT_r[:, 8 * hf:8 * hf + 8,
                                         n * 512:(n + 1) * 512])
                        xah.append(xa)

                    if n == 0:
                        # remaining weights queue behind the first-matmul
                        # critical loads, ordered by first compute use
                        for qk in range(1, 4):
                            nc.sync.dma_start(
                                wqn[:, 4 * qk:4 * qk + 4],
                                wq_r[:, 4 * qk:4 * qk + 4, 0:512])
                        nc.sync.dma_start(wqr, wq_r[:, :, 512:768])
                        nc.sync.dma_start(
                            cos_all,
                            cos_d[:].rearrange("(tb p) i -> p tb i", p=P))
                        nc.sync.dma_start(
                            sin_all,
                            sin_d[:].rearrange("(tb p) i -> p tb i", p=P))
                    if n == 1:
                        # phase-B weights prefetch behind phase-A's
                        for wk in range(4):
                            nc.sync.dma_start(
                                wckv_sb[:, 4 * wk:4 * wk + 4],
                                wckv_r[:, 4 * wk:4 * wk + 4])
                    if n == 2:
                        # phase-C weights, needed later still
                        nc.sync.dma_start(
                            wdn, wdn_d[:].rearrange("(kc p) m -> p kc m",
                                                    p=P))
                        nc.sync.dma_start(
                            wdv, wdv_d[:].rearrange("(kc p) m -> p kc m",
                                                    p=P))
                    if n == 3:
                        # phase B's first x-slice overlaps phase-A tail
                        xb_pre = load_xb(0)

                    def xat(k):
                        return xah[k // 8][:, k % 8]

                    for m in range(4):
                        for k in range(KC):
                            nc.tensor.matmul(
                                pn[m], r(wqn[:, k, m * 128:(m + 1) * 128]),
                                r(xat(k)), start=(k == 0), stop=(k == KC - 1))
                    for s in range(4):
                        for k in range(KC):
                            nc.tensor.matmul(
                                pr[s], r(xat(k)[:, s * 128:(s + 1) * 128]),
                                r(wqr[:, k, :]),
                                start=(k == 0), stop=(k == KC - 1))
                    for m in range(4):
                        nc.scalar.copy(
                            qnopeT[m][:, n * 512:(n + 1) * 512], pn[m])
                    for s in range(4):
                        tb = n * 4 + s
                        cosv = cos_all[:, tb][:, None, :].to_broadcast(
                            (P, 4, 32))
                        sinv = sin_all[:, tb][:, None, :].to_broadcast(
                            (P, 4, 32))
                        prv = pr[s].rearrange("p (g i) -> p g i", i=64)
                        qe, qo = prv[:, :, 0:32], prv[:, :, 32:64]
                        ta = stA.tile([P, 128], f32, tag="ta",
                                      name="ta").rearrange(
                            "p (g i) -> p g i", i=32)
                        tb_ = stA.tile([P, 128], f32, tag="tb",
                                       name="tb").rearrange(
                            "p (g i) -> p g i", i=32)
                        tc2 = stA.tile([P, 128], f32, tag="tc",
                                       name="tc").rearrange(
                            "p (g i) -> p g i", i=32)
                        td = stA.tile([P, 128], f32, tag="td",
                                      name="td").rearrange(
                            "p (g i) -> p g i", i=32)
                        qvv = qrotS[:, tb].rearrange("p (g i) -> p g i",
                                                     i=64)
                        nc.vector.tensor_tensor(ta, qe, cosv, AL.mult)
                        nc.vector.tensor_tensor(tb_, qo, sinv, AL.mult)
                        nc.vector.tensor_tensor(qvv[:, :, 0:32], ta, tb_,
                                                AL.subtract)
                        nc.vector.tensor_tensor(tc2, qo, cosv, AL.mult)
                        nc.vector.tensor_tensor(td, qe, sinv, AL.mult)
                        nc.vector.tensor_tensor(qvv[:, :, 32:64], tc2, td,
                                                AL.add)

            # ---- phase B: ckv -> rms/rope -> transposed tensors ----------
            if True:
                with tc.tile_pool(name="stB", bufs=3) as stB, \
                     tc.tile_pool(name="smB", bufs=4) as smB:
                    tr_idx = [0]
                    for n in range(8):  # 256-token slices
                        pcs = [[psu.tile([P, 288], f32,
                                         tag=f"bk{2 * s_ + u}",
                                         name=f"ckv{u}")
                                for u in range(2)] for s_ in range(2)]
                        xbh = xb_pre if n == 0 else load_xb(n)

                        def xbt(k):
                            return xbh[k // 8][:, k % 8]

                        for s in range(2):
                            for u in range(2):
                                wsl = (slice(0, 288), slice(288, 576))[u]
                                for k in range(KC):
                                    nc.tensor.matmul(
                                        pcs[s][u],
                                        r(xbt(k)[:, s * 128:(s + 1) * 128]),
                                        r(wckv_sb[:, k, wsl]),
                                        start=(k == 0), stop=(k == KC - 1))
                        for s in range(2):
                            tb = n * 2 + s
                            p0, p1_ = pcs[s]
                            sq = stB.tile([P, 288], f32, tag="sq", name="sq")
                            sq2 = stB.tile([P, 224], f32, tag="sq2",
                                           name="sq2")
                            ss0 = smB.tile([P, 1], f32, tag="ss0", name="ss0")
                            ss1 = smB.tile([P, 1], f32, tag="ss1", name="ss1")
                            nc.scalar.activation(sq, p0, AF.Square,
                                                 accum_out=ss0)
                            nc.scalar.activation(sq2, p1_[:, 0:224],
                                                 AF.Square, accum_out=ss1)
                            ssum = smB.tile([P, 1], f32, tag="ss", name="ss")
                            nc.vector.tensor_add(ssum, ss0, ss1)
                            lnv = smB.tile([P, 1], f32, tag="lnv", name="lnv")
                            nc.scalar.activation(lnv, ssum, AF.Ln,
                                                 bias=eps_t,
                                                 scale=1.0 / LORA)
                            rfac = smB.tile([P, 1], f32, tag="rfac",
                                            name="rfac")
                            nc.scalar.activation(rfac, lnv, AF.Exp,
                                                 scale=-0.5)
                            ckvn = stB.tile([P, 512], f32r, tag="ckvn",
                                            name="ckvn")
                            nc.scalar.mul(ckvn[:, 0:288], p0, rfac)
                            nc.scalar.mul(ckvn[:, 288:512], p1_[:, 0:224],
                                          rfac)
                            # k_rope rotation (raw latent, un-normalized)
                            ke, ko = p1_[:, 224:256], p1_[:, 256:288]
                            cosv, sinv = cos_all[:, tb], sin_all[:, tb]
                            ra = stB.tile([P, 32], f32, tag="ra", name="ra")
                            rb = stB.tile([P, 32], f32, tag="rb", name="rb")
                            rc = stB.tile([P, 32], f32, tag="rc", name="rc")
                            rd = stB.tile([P, 32], f32, tag="rd", name="rd")
                            krt = stB.tile([P, 64], f32r, tag="krt",
                                           name="krt")
                            nc.vector.tensor_tensor(ra, ke, cosv, AL.mult)
                            nc.vector.tensor_tensor(rb, ko, sinv, AL.mult)
                            nc.vector.tensor_tensor(krt[:, 0:32], ra, rb,
                                                    AL.subtract)
                            nc.vector.tensor_tensor(rc, ko, cosv, AL.mult)
                            nc.vector.tensor_tensor(rd, ke, sinv, AL.mult)
                            nc.vector.tensor_tensor(krt[:, 32:64], rc, rd,
                                                    AL.add)
                            # transposes -> persistent ^T tensors
                            tcol = slice(tb * 128, (tb + 1) * 128)
                            for dc in range(4):
                                pt = psu.tile([P, P], f32r,
                                              tag=f"bk{4 + tr_idx[0] % 4}",
                                              name="tr")
                                tr_idx[0] += 1
                                nc.tensor.transpose(
                                    pt, ckvn[:, dc * 128:(dc + 1) * 128],
                                    ident)
                                nc.vector.tensor_copy(ckvT[dc][:, tcol], pt)
                            pt = psu.tile([P, P], f32r,
                                          tag=f"bk{4 + tr_idx[0] % 4}",
                                          name="tr")
                            tr_idx[0] += 1
                            nc.tensor.transpose(pt[0:64, :], krt, ident)
                            nc.vector.tensor_copy(kz[0][0:64, tcol],
                                                  pt[0:64, :])
                            qr_in = qrotS[:, tb]
                            for pc in range(2):
                                pt = psu.tile([P, P], f32r,
                                              tag=f"bk{4 + tr_idx[0] % 4}",
                                              name="tr")
                                tr_idx[0] += 1
                                nc.tensor.transpose(
                                    pt, qr_in[:, pc * 128:(pc + 1) * 128],
                                    ident)
                                nc.vector.tensor_copy(qropeT[pc][:, tcol], pt)

                # odd heads' row-group: shifted copy into the upper half
                nc.sync.dma_start(kz[1][64:128, :], kz[0][0:64, :])

                # phase-D mask tiles prefetch while phase C computes
                mg_all = {}
                for g in range(n_generic):
                    mt = const.tile([P, P], f32, tag=f"mg{g}", name=f"mg{g}")
                    nc.sync.dma_start(mt, maskg_d[g])
                    mg_all[g] = mt

                # ---- phase C: k_nope^T per head, v (SBUF) ---------------
                for h in range(4):
                    for n4 in range(4):
                        pk = psu.tile([P, 512], f32,
                                      tag=f"bk{(h * 4 + n4) % 4}",
                                      name="kn")
                        for kc in range(4):
                            nc.tensor.matmul(
                                pk, r(wdn[:, kc, h * 128:(h + 1) * 128]),
                                r(ckvT[kc][:, n4 * 512:(n4 + 1) * 512]),
                                start=(kc == 0), stop=(kc == 3))
                        nc.vector.tensor_copy(
                            knopeT[h][:, n4 * 512:(n4 + 1) * 512], pk)
                for tb in range(TB):
                    pv = psu.tile([P, 512], f32,
                                  tag=f"bk{4 + tb % 4}", name="v")
                    for kc in range(4):
                        nc.tensor.matmul(
                            pv, r(ckvT[kc][:, tb * P:(tb + 1) * P]),
                            r(wdv[:, kc, :]),
                            start=(kc == 0), stop=(kc == 3))
                    nc.vector.tensor_copy(vS[:, tb], pv)

            xb_cm.__exit__(None, None, None)
            pckvT_cm.__exit__(None, None, None)
            psu_cm.__exit__(None, None, None)

            # ---- phase D: attention + proj -------------------------------
            with tc.tile_pool(name="phD", bufs=1) as pD, \
                 tc.tile_pool(name="sp", bufs=18) as sp, \
                 tc.tile_pool(name="stD", bufs=3) as stD, \
                 tc.tile_pool(name="attp", bufs=2) as attp, \
                 tc.tile_pool(name="psS", bufs=3, space="PSUM") as psS, \
                 tc.tile_pool(name="psAtt", bufs=2, space="PSUM") as psAtt, \
                 tc.tile_pool(name="psL", bufs=1, space="PSUM") as psL, \
                 tc.tile_pool(name="psO", bufs=2, space="PSUM") as psO:
                wproj_sb = [pD.tile([P, C], f32r, tag=f"wp{h}",
                                    name=f"wp{h}") for h in range(4)]
                for h in range(4):
                    nc.sync.dma_start(wproj_sb[h],
                                      wproj_d[h * P:(h + 1) * P, :])
                def emit_proj(jp, att_p):
                    for qs in range(4):
                        for ct in range(4):
                            pso = psO.tile([P, 512], f32, tag="o",
                                            name="o")
                            for h in range(4):
                                nc.tensor.matmul(
                                    pso,
                                    r(att_p[h][:, qs * 128:(qs + 1) * 128]),
                                    r(wproj_sb[h][:,
                                                  ct * 512:(ct + 1) * 512]),
                                    start=(h == 0), stop=(h == 3))
                            ost = sp.tile([P, 512], f32r, tag="ost",
                                          name="ost", bufs=3)
                            # split PSUM read across both engines so the
                            # bank frees sooner for the next proj group
                            nc.vector.tensor_copy(ost[:, 0:256],
                                                  pso[:, 0:256])
                            nc.scalar.copy(ost[:, 256:512],
                                           pso[:, 256:512])
                            nc.sync.dma_start(
                                out_d[512 * jp + 128 * qs:
                                      512 * jp + 128 * (qs + 1),
                                      ct * 512:(ct + 1) * 512], ost)

                prev_proj = None
                for j in range(NQ):
                    chunks = plan[j]
                    nchunks = len(chunks)
                    attT = {}
                    for g in range(4):  # heads, pipelined sequentially
                        hs = (g,)
                        st = {}
                        for h in hs:
                            ps_att = psAtt.tile([P, 512], f32, tag="att",
                                                name="att")
                            ps_l = psL.tile([P, 512], f32, tag="l",
                                             name="l")
                            st[h] = dict(att=ps_att, l=ps_l)

                        def scores_mm(h, ci):
                            c, col0, sub = chunks[ci]
                            hp, pr_ = h % 2, h // 2
                            qsl = slice(512 * j + col0, 512 * (j + 1))
                            kcl = slice(128 * c, 128 * (c + 1))
                            ps_s = psS.tile([P, 512], f32, tag="s", name="s")
                            nc.tensor.matmul(
                                ps_s[:, col0:], r(knopeT[h][:, kcl]),
                                r(qnopeT[h][:, 512 * j + col0:
                                            512 * (j + 1)]),
                                start=True, stop=False)
                            nc.tensor.matmul(
                                ps_s[:, col0:],
                                r(kz[hp][:, kcl]),
                                r(qropeT[pr_][:, qsl]),
                                start=False, stop=True)
                            return ps_s

                        def exp_mask(h, ci, ps_s):
                            c, col0, sub = chunks[ci]
                            sprime = sp.tile([P, 512], f32r, tag="sp",
                                             name="sp")
                            nc.scalar.activation(
                                sprime[:, col0:], ps_s[:, col0:],
                                AF.Exp, scale=SCALE)
                            for qs, s in enumerate(sub):
                                colA, colB = 128 * qs, 128 * (qs + 1)
                                if colA < col0 or s == "zero":
                                    continue
                                if s == "skip":
                                    zv = sprime[:, colA:colB]
                                    if MM_DTYPE == "float32r":
                                        zv = zv.bitcast(f32)
                                    nc.any.memset(zv, 0.0)
                                else:
                                    mt = mg_all[s[1]]
                                    stt = stD.tile([P, P], f32, tag="stt",
                                                   name="stt")
                                    nc.vector.scalar_tensor_tensor(
                                        stt, ps_s[:, colA:colB], SCALE, mt,
                                        AL.mult, AL.add)
                                    nc.scalar.activation(
                                        sprime[:, colA:colB], stt, AF.Exp,
                                        scale=1.0)
                            return sprime

                        def att_mm(h, ci, sprime):
                            c, col0, sub = chunks[ci]
                            nc.tensor.matmul(
                                st[h]["att"][:, col0:],
                                r(vS[:, c, h * 128:(h + 1) * 128]),
                                r(sprime[:, col0:]),
                                start=(ci == 0), stop=(ci == nchunks - 1))

                        h = hs[0]
                        pend = [scores_mm(h, 0)]
                        if nchunks > 1:
                            pend.append(scores_mm(h, 1))
                        sprimes = []
                        for ci in range(nchunks):
                            sprime = exp_mask(h, ci, pend[ci])
                            sprimes.append(sprime)
                            if ci + 2 < nchunks:
                                pend.append(scores_mm(h, ci + 2))
                            att_mm(h, ci, sprime)
                        # deferred row-sum burst: dense back-to-back matmuls,
                        # no per-chunk ACT waits on the PE stream
                        for ci, sprime in enumerate(sprimes):
                            col0 = chunks[ci][1]
                            nc.tensor.matmul(
                                st[h]["l"][:, col0:], r(ones128),
                                r(sprime[:, col0:]),
                                start=(ci == 0), stop=(ci == nchunks - 1))
                        for h in hs:
                            lnl = stD.tile([P, 512], f32, tag="lr",
                                           name="lr")
                            nc.scalar.activation(lnl, st[h]["l"], AF.Ln)
                            rec = stD.tile([P, 512], f32, tag="lr",
                                           name="lr")
                            nc.scalar.activation(rec, lnl, AF.Exp,
                                                 scale=-1.0)
                            at = attp.tile([P, 512], f32r, tag=f"at{h}",
                                           name=f"at{h}")
                            nc.vector.tensor_tensor(at, st[h]["att"], rec,
                                                    AL.mult)
                            attT[h] = at
                    emit_proj(j, attT)

    orig_tables = bacc.get_activation_tables
    bacc.get_activation_tables = _act_tables_combined_only
    try:
        nc.compile()
    finally:
        bacc.get_activation_tables = orig_tables
    return nc


# ---------------------------------------------------------------- entry

def _ensure_axon_hook_shim():
    # bass_utils imports antenv.axon_hooks when tracing is requested via
    # env; provide a null hook module if the image lacks it so kernel()
    # never crashes on that path.
    try:
        import antenv.axon_hooks  # noqa: F401
    except Exception:
        import sys
        import types
        m = types.ModuleType("antenv.axon_hooks")
        _h = [None]
        m.set_axon_ntff_profile_hook = lambda h: _h.__setitem__(0, h)
        m.get_axon_ntff_profile_hook = lambda: _h[0]
        sys.modules["antenv.axon_hooks"] = m
        try:
            import antenv
            antenv.axon_hooks = m
        except Exception:
            pass


def kernel(x, freq_cis, mask, window, Wq, Wckv, kv_norm_w, Wdkv, Wproj,
           start_pos):
    global LAST_RESULTS
    _ensure_axon_hook_shim()
    from concourse.bass_utils import run_bass_kernel_spmd

    x = np.asarray(x, np.float32)
    freq_cis = np.asarray(freq_cis, np.float32)
    mask = np.asarray(mask, np.float32)
    Wq = np.asarray(Wq, np.float32)
    Wckv = np.asarray(Wckv, np.float32)
    kv_norm_w = np.asarray(kv_norm_w, np.float32)
    Wdkv = np.asarray(Wdkv, np.float32)
    Wproj = np.asarray(Wproj, np.float32)

    plan, maskg = _mask_plan(mask)
    key = (MM_DTYPE, _plan_key(plan))
    if key not in _prog_cache:
        _prog_cache[key] = _build(plan, maskg.shape[0])
    nc = _prog_cache[key]

    cosT = np.ascontiguousarray(freq_cis[:, :, 0])
    sinT = np.ascontiguousarray(freq_cis[:, :, 1])
    wckv_p = _pack_wckv(Wckv)

    in_maps = []
    for core in range(N_CORES):
        b, hg = core // 4, core % 4
        wdn, wdv = _pack_wdkv(Wdkv, kv_norm_w, hg)
        in_maps.append({
            "xT": np.ascontiguousarray(x[b].T),
            "wq": _pack_wq(Wq, hg),
            "wckv": wckv_p,
            "wdn": wdn,
            "wdv": wdv,
            "wproj": np.ascontiguousarray(Wproj[hg * 512:(hg + 1) * 512, :]),
            "cosT": cosT,
            "sinT": sinT,
            "maskg": maskg,
        })

    if MM_DTYPE == "bfloat16":
        import ml_dtypes
        mmdt = ml_dtypes.bfloat16
        for m in in_maps:
            for k in ("xT", "wq", "wckv", "wdn", "wdv", "wproj"):
                m[k] = m[k].astype(mmdt)

    res = run_bass_kernel_spmd(nc, in_maps, list(range(N_CORES)))
    LAST_RESULTS = res
    outs = [np.asarray(res.results[c]["out"], np.float32)
            for c in range(N_CORES)]
    full = np.empty((B, T, C), np.float32)
    for b in range(B):
        full[b] = outs[4 * b] + outs[4 * b + 1] + outs[4 * b + 2] \
            + outs[4 * b + 3]
    return full

